# revision 1
# baseline (speedup 1.0000x reference)
"""DevignLite GNN (3-layer GCN + dual pooling + MLP head) on 8 Trainium2 NeuronCores.

Strategy
--------
- Nodes (and their incident edges, bucketed by dst) are partitioned across the
  8 cores.  Per GCN layer, using the separable GCN normalization
  norm(e) = dinv[src] * dinv[dst]:
      u = dinv * x                     (node-local scale, fused into table write)
      z[v] = sum_{e: dst=v} u[src_e]   (pure 0/1 aggregation incl. self loops)
      x' = relu((dinv * z) @ W + b)
- The per-edge gather u[src] is a SWDGE `dma_gather` (256B f32 rows, int16
  indices into quarter-major shard views of the replicated table).
- Segment-sum by dst on the PE: per 128-edge tile, a one-hot matrix S (DVE
  iota + is_equal from shipped local-dst ids) is lhsT: psum[dst,64] += S.T@msg.
- Per-layer u replication via 4 quarter shared-output AllGathers, interleaved
  with the per-node update loop so the next layer's gathers on quarter q
  overlap the remaining quarters' collectives.
- Pooling: segment mean via one-hot matmul with a ones column; segment max via
  a segmented max-scan (Hillis-Steele along free dim) + one-hot extraction of
  segment-end columns.  Partials exchanged with a small AllGather; every core
  computes the classifier head identically.

The schedule is identical on all 8 cores (SPMD); per-core variation lives in
input tensor data only (index streams, one-hot ids, degree vectors).
"""

import sys

sys.path.insert(0, "/opt/trn_rl_repo")

import numpy as np
import ml_dtypes

P = 128
D = 64


class Cfg:
    def __init__(self, N, E, V, G, n_cores=8, call_tiles=64):
        self.N, self.E, self.V, self.G = N, E, V, G
        self.NC = n_cores
        assert N % n_cores == 0
        self.NL = N // n_cores                      # nodes per core
        self.NLP = -(-self.NL // P) * P             # padded to 128
        self.DB = self.NLP // P                     # dst blocks per core
        # quarter-major shard views: split each rank's rows into NQ quarters
        # (tile-aligned); shard q's table = all ranks' quarter-q rows.
        NQ = 4
        base = self.DB // NQ
        rem = self.DB % NQ
        qtiles = [base + (1 if i < rem else 0) for i in range(NQ)]
        self.qtiles = [q for q in qtiles if q > 0]
        self.NSH = len(self.qtiles)
        self.qrows = [q * P for q in self.qtiles]
        self.qstart = [0]
        for q in self.qrows[:-1]:
            self.qstart.append(self.qstart[-1] + q)
        for q in self.qrows:
            assert n_cores * q < 32768, "quarter shard exceeds int16 range" 
        self.TROW = n_cores * self.NLP              # rows of full u table
        self.VBS = min(V, 25000)                    # vocab block size
        self.NVB = -(-V // self.VBS)
        self.GB = -(-G // P)                        # graph blocks (pool)
        self.CALL_TILES = call_tiles


# ----------------------------------------------------------------------------
# host-side preprocessing (structure only: bucketing, index streams, degrees)
# ----------------------------------------------------------------------------
def _preprocess(cfg, x_tokens, edge_index, batch):
    c = cfg
    N, NC, NL, NLP, DB, NSH = c.N, c.NC, c.NL, c.NLP, c.DB, c.NSH
    bf16 = ml_dtypes.bfloat16

    src = np.asarray(edge_index[0], dtype=np.int64)
    dst = np.asarray(edge_index[1], dtype=np.int64)
    loop = np.arange(N, dtype=np.int64)
    src = np.concatenate([src, loop])
    dst = np.concatenate([dst, loop])
    deg = np.bincount(dst, minlength=N).astype(np.float32)

    src_rank = src // NL
    src_loc = src % NL
    qstart_arr = np.asarray(c.qstart, dtype=np.int64)
    qrows_arr = np.asarray(c.qrows, dtype=np.int64)
    shard = np.searchsorted(qstart_arr, src_loc, side="right") - 1
    idx16 = (src_rank * qrows_arr[shard] + (src_loc - qstart_arr[shard])).astype(np.int64)
    edge_core = dst // NL
    ldst = dst % NL
    db = ldst // P
    lid = ldst % P
    cell = shard * DB + db

    NCELL = NSH * DB
    counts = np.zeros((NC, NCELL), dtype=np.int64)
    percore = []
    for ci in range(NC):
        m = edge_core == ci
        cc = cell[m]
        order = np.argsort(cc, kind="stable")
        counts[ci] = np.bincount(cc, minlength=NCELL)
        percore.append((cc[order], idx16[m][order], lid[m][order]))

    slot = -(-counts.max(axis=0) // P)               # tiles per cell, shared
    tiles_per_shard = slot.reshape(NSH, DB).sum(axis=1)
    cell_tile_start = np.concatenate([[0], np.cumsum(slot)[:-1]])
    NT_TOT = int(slot.sum())

    calls = []                                       # (shard, tile0, ntiles)
    t0 = 0
    for s in range(NSH):
        rem = int(tiles_per_shard[s])
        base = t0
        off = 0
        while rem > 0:
            nt = min(c.CALL_TILES, rem)
            calls.append((s, base + off, nt))
            off += nt
            rem -= nt
        t0 = base + int(tiles_per_shard[s])
    cells_sched = []                                 # (s, db, tile0, ntiles, first)
    first_seen = set()
    for s in range(NSH):
        for d in range(DB):
            sl = int(slot[s * DB + d])
            if sl == 0:
                continue
            first = d not in first_seen
            first_seen.add(d)
            cells_sched.append((s, d, int(cell_tile_start[s * DB + d]), sl, first))

    edge_idx_all = np.zeros((NC, NT_TOT * P), dtype=np.int16)
    edge_ids_all = np.full((NC, NT_TOT * P), -1.0, dtype=np.float32)
    for ci in range(NC):
        cc, ii, ll = percore[ci]
        within = np.arange(cc.size) - np.concatenate(
            [[0], np.cumsum(counts[ci])[:-1]]
        )[cc]
        pos = cell_tile_start[cc] * P + within
        edge_idx_all[ci, pos] = ii.astype(np.int16)
        edge_ids_all[ci, pos] = ll.astype(np.float32)

    def wrap_idx(a):                                 # [n] -> [128, n/16] int16
        n = a.size
        assert n % 16 == 0
        w = a.reshape(n // 16, 16).T
        return np.tile(w, (8, 1)).astype(np.int16)

    def tile_layout(a, fill, ncols):                 # [n] -> [128, ncols]
        out = np.full((P, ncols), fill, dtype=np.float32)
        n = a.size
        t = np.arange(n) // P
        p = np.arange(n) % P
        out[p, t] = a
        return out

    edge_idx_w = np.stack([wrap_idx(edge_idx_all[ci]) for ci in range(NC)])
    edge_ids_t = np.stack(
        [tile_layout(edge_ids_all[ci], -1.0, NT_TOT) for ci in range(NC)]
    )

    # --- embedding gather / scatter streams -------------------------------
    toks = np.asarray(x_tokens, dtype=np.int64).reshape(-1)
    vb = toks // c.VBS
    emb_cnt = np.zeros((NC, c.NVB), dtype=np.int64)
    for ci in range(NC):
        emb_cnt[ci] = np.bincount(vb[ci * NL : (ci + 1) * NL], minlength=c.NVB)
    EC = int(-(-emb_cnt.max() // P) * P)
    TRASH = NLP
    tok_idx = np.zeros((NC, c.NVB * EC), dtype=np.int16)
    tok_scat = np.full((NC, c.NVB * EC), TRASH, dtype=np.int16)
    deg_perm = np.ones((NC, c.NVB * EC), dtype=np.float32)
    for ci in range(NC):
        tl = toks[ci * NL : (ci + 1) * NL]
        dl = deg[ci * NL : (ci + 1) * NL]
        vbl = vb[ci * NL : (ci + 1) * NL]
        for b in range(c.NVB):
            rows = np.nonzero(vbl == b)[0]
            o = b * EC
            tok_idx[ci, o : o + rows.size] = (tl[rows] % c.VBS).astype(np.int16)
            tok_scat[ci, o : o + rows.size] = rows.astype(np.int16)
            deg_perm[ci, o : o + rows.size] = dl[rows]

    tok_idx_w = np.stack([wrap_idx(tok_idx[ci]) for ci in range(NC)])
    tok_scat_w = np.stack([wrap_idx(tok_scat[ci]) for ci in range(NC)])
    ECC = EC // P
    deg_perm_t = np.stack(
        [
            np.concatenate(
                [
                    tile_layout(deg_perm[ci, b * EC : (b + 1) * EC], 1.0, ECC)
                    for b in range(c.NVB)
                ],
                axis=1,
            )
            for ci in range(NC)
        ]
    )

    # --- per-node degree / graph metadata ---------------------------------
    batch = np.asarray(batch, dtype=np.int64)
    deg_loc = np.ones((NC, P, DB), dtype=np.float32)
    g_ids = np.full((NC, P, DB), -1.0, dtype=np.float32)
    is_end = np.zeros((NC, P, DB), dtype=np.float32)
    for ci in range(NC):
        dl = deg[ci * NL : (ci + 1) * NL]
        bl = batch[ci * NL : (ci + 1) * NL].astype(np.float32)
        e = np.zeros(NL, dtype=np.float32)
        if NL > 1:
            e[:-1] = (bl[1:] != bl[:-1]).astype(np.float32)
        e[-1] = 1.0
        deg_loc[ci] = tile_layout(dl, 1.0, DB)
        g_ids[ci] = tile_layout(bl, -1.0, DB)
        is_end[ci] = tile_layout(e, 0.0, DB)

    meta = dict(calls=calls, cells=cells_sched, NT_TOT=NT_TOT, EC=EC)
    data = dict(
        edge_idx=edge_idx_w,
        edge_ids=edge_ids_t,
        tok_idx=tok_idx_w,
        tok_scat=tok_scat_w,
        deg_perm=deg_perm_t,
        deg_loc=deg_loc,
        g_ids=g_ids,
        is_end=is_end,
    )
    return meta, data


# ----------------------------------------------------------------------------
# the Bass/Tile program
# ----------------------------------------------------------------------------
def build_program(cfg, meta):
    import concourse.bacc as bacc
    import concourse.tile as tile
    from concourse import mybir
    from concourse.masks import make_identity

    c = cfg
    f32 = mybir.dt.float32
    bf16 = mybir.dt.bfloat16
    i16 = mybir.dt.int16
    AF = mybir.ActivationFunctionType
    OP = mybir.AluOpType
    NT_TOT, EC, CALLS, CELLS = meta["NT_TOT"], meta["EC"], meta["calls"], meta["cells"]
    ECC = EC // P
    DBL = c.DB
    rg = [list(range(c.NC))]
    PCOLS = 2 * (2 * D + 1)                          # per-graph-block pool cols

    nc = bacc.Bacc("TRN2", target_bir_lowering=False, debug=False,
                   enable_asserts=False, num_devices=c.NC)

    emb = nc.dram_tensor("emb_table", [c.V, D], f32, kind="ExternalInput")
    edge_idx_d = nc.dram_tensor("edge_idx", [P, NT_TOT * 8], i16, kind="ExternalInput")
    edge_ids_d = nc.dram_tensor("edge_ids", [P, NT_TOT], f32, kind="ExternalInput")
    tok_idx_d = nc.dram_tensor("tok_idx", [P, c.NVB * EC // 16], i16, kind="ExternalInput")
    tok_scat_d = nc.dram_tensor("tok_scat", [P, c.NVB * EC // 16], i16, kind="ExternalInput")
    deg_perm_d = nc.dram_tensor("deg_perm", [P, c.NVB * ECC], f32, kind="ExternalInput")
    deg_loc_d = nc.dram_tensor("deg_loc", [P, DBL], f32, kind="ExternalInput")
    g_ids_d = nc.dram_tensor("g_ids", [P, DBL], f32, kind="ExternalInput")
    is_end_d = nc.dram_tensor("is_end", [P, DBL], f32, kind="ExternalInput")
    Ws = [nc.dram_tensor(f"W{i}", [D, D], f32, kind="ExternalInput") for i in range(3)]
    bs = [nc.dram_tensor(f"b{i}", [D], f32, kind="ExternalInput") for i in range(3)]
    Wc1_d = nc.dram_tensor("Wc1", [2 * D, D], f32, kind="ExternalInput")
    bc1_d = nc.dram_tensor("bc1", [D], f32, kind="ExternalInput")
    Wc2_d = nc.dram_tensor("Wc2", [D, 2], f32, kind="ExternalInput")
    bc2_d = nc.dram_tensor("bc2", [2], f32, kind="ExternalInput")
    logits_d = nc.dram_tensor("logits", [c.G, 2], f32, kind="ExternalOutput")

    u_loc = [
        nc.dram_tensor(f"u{i}_loc", [c.NLP + P, D], f32, kind="Internal")
        for i in range(3)
    ]
    u_full = [
        [
            nc.dram_tensor(f"u{i}_full_q{q}", [c.NC * c.qrows[q], D], f32,
                           kind="Internal", addr_space="Shared")
            for q in range(c.NSH)
        ]
        for i in range(3)
    ]
    pool_loc_d = nc.dram_tensor("pool_loc", [P, c.GB * PCOLS], f32, kind="Internal")
    pool_all_d = nc.dram_tensor("pool_all", [c.NC * P, c.GB * PCOLS], f32,
                                kind="Internal", addr_space="Shared")

    iota_f = nc.inline_tensor(
        np.tile(np.arange(P, dtype=np.float32), (P, 1)), name="iota_f"
    )

    def rsqrt_refined(dst, deg_src, tmp_pool, ncols):
        """dst = deg_src**-0.5 with one Newton step (f32-accurate)."""
        nc.scalar.sqrt(dst[:], deg_src[:])
        nc.vector.reciprocal(dst[:], dst[:])
        t = tmp_pool.tile([P, ncols], f32, tag="nwt", name=f"nwt{id(dst) % 9999}")
        nc.vector.tensor_tensor(t[:], dst[:], dst[:], OP.mult)
        nc.vector.tensor_tensor(t[:], t[:], deg_src[:], OP.mult)
        nc.vector.tensor_scalar(t[:], t[:], -0.5, 1.5, OP.mult, OP.add)
        nc.vector.tensor_tensor(dst[:], dst[:], t[:], OP.mult)

    with tile.TileContext(nc) as tc:
        with (
            tc.tile_pool(name="persist", bufs=1) as pp,
            tc.tile_pool(name="msg", bufs=5) as msgp,
            tc.tile_pool(name="sel", bufs=8) as sp,
            tc.tile_pool(name="work", bufs=2) as wp,
            tc.tile_pool(name="ps", bufs=4, space="PSUM") as psp,
            tc.tile_pool(name="ppool", bufs=1, space="PSUM") as ppsum,
        ):
            # ---------- persistent SBUF state --------------------------------
            idx_sb = pp.tile([P, NT_TOT * 8], i16, tag="idx")
            nc.sync.dma_start(idx_sb[:], edge_idx_d[:])
            ids_sb = pp.tile([P, NT_TOT], f32, tag="ids")
            nc.sync.dma_start(ids_sb[:], edge_ids_d[:])
            tok_idx_sb = pp.tile([P, c.NVB * EC // 16], i16, tag="tokidx")
            nc.sync.dma_start(tok_idx_sb[:], tok_idx_d[:])
            tok_scat_sb = pp.tile([P, c.NVB * EC // 16], i16, tag="tokscat")
            nc.sync.dma_start(tok_scat_sb[:], tok_scat_d[:])
            iota_f_sb = pp.tile([P, P], f32, tag="iotaf")
            nc.sync.dma_start(iota_f_sb[:], iota_f[:])
            ident = pp.tile([P, P], f32, tag="ident")
            make_identity(nc, ident[:])
            g_ids_sb = pp.tile([P, DBL], f32, tag="gids")
            nc.sync.dma_start(g_ids_sb[:], g_ids_d[:])
            is_end_sb = pp.tile([P, DBL], f32, tag="iend")
            nc.sync.dma_start(is_end_sb[:], is_end_d[:])
            ones_row = pp.tile([1, D], f32, tag="ones_row")
            nc.vector.memset(ones_row[:], 1.0)


            deg_sb = wp.tile([P, DBL], f32, tag="deg")
            nc.sync.dma_start(deg_sb[:], deg_loc_d[:])
            dinv = pp.tile([P, DBL], f32, tag="dinv")
            rsqrt_refined(dinv, deg_sb, wp, DBL)
            degp_sb = wp.tile([P, c.NVB * ECC], f32, tag="degp")
            nc.sync.dma_start(degp_sb[:], deg_perm_d[:])
            dinvp = pp.tile([P, c.NVB * ECC], f32, tag="dinvp")
            rsqrt_refined(dinvp, degp_sb, wp, c.NVB * ECC)

            W_sb, b_sb = [], []
            for i in range(3):
                w = pp.tile([D, D], f32, tag=f"W{i}")
                nc.sync.dma_start(w[:], Ws[i][:])
                W_sb.append(w)
                b = pp.tile([D, 1], f32, tag=f"b{i}")
                nc.sync.dma_start(b[:], bs[i][:, None])
                b_sb.append(b)
            Wc1_sb = pp.tile([2 * D, D], f32, tag="Wc1")
            nc.sync.dma_start(Wc1_sb[:], Wc1_d[:])
            bc1_sb = pp.tile([D, 1], f32, tag="bc1")
            nc.sync.dma_start(bc1_sb[:], bc1_d[:, None])
            Wc2_sb = pp.tile([D, 2], f32, tag="Wc2")
            nc.sync.dma_start(Wc2_sb[:], Wc2_d[:])
            bc2_sb = pp.tile([2, 1], f32, tag="bc2")
            nc.sync.dma_start(bc2_sb[:], bc2_d[:, None])

            z_acc = pp.tile([P, DBL * D + D], f32, tag="zacc")

            # ---------- embedding: u0 = dinv * emb[tok] ----------------------
            nc.vector.memset(z_acc[:], 0.0)
            zcols = (c.NLP + P) * D // P
            nc.gpsimd.dma_start(
                u_loc[0][:, :].rearrange("(a b) c -> a (b c)", a=P),
                z_acc[:, 0:zcols],
            )
            for b in range(c.NVB):
                g = msgp.tile([P, ECC, D], f32, tag="msg")
                nc.gpsimd.dma_gather(
                    g[:], emb[b * c.VBS : min((b + 1) * c.VBS, c.V), :],
                    tok_idx_sb[:, b * (EC // 16) : (b + 1) * (EC // 16)],
                    EC, EC, D, elem_step=D, single_packet=False,
                )
                sc = msgp.tile([P, ECC, D], f32, tag="msg")
                for cc in range(ECC):
                    nc.vector.tensor_scalar(
                        sc[:, cc, 0:D], g[:, cc, :],
                        dinvp[:, b * ECC + cc : b * ECC + cc + 1], None, OP.mult,
                    )
                nc.gpsimd.dma_scatter_add(
                    u_loc[0][:, :], sc[:],
                    tok_scat_sb[:, b * (EC // 16) : (b + 1) * (EC // 16)],
                    EC, EC, D, elem_step=D, single_packet=False,
                )
            for q in range(c.NSH):
                nc.gpsimd.collective_compute(
                    "AllGather", OP.bypass, replica_groups=rg,
                    ins=[u_loc[0][c.qstart[q] : c.qstart[q] + c.qrows[q], :]],
                    outs=[u_full[0][q][:, :]],
                )

            # ---------- GCN layers -------------------------------------------
            pool_carry = {"g": None, "v": None}
            pool_sum_ps = [
                ppsum.tile([P, D + 1], f32, tag=f"pls{g}", name=f"pls{g}")
                for g in range(c.GB)
            ]
            pool_max_ps = [
                ppsum.tile([P, D], f32, tag=f"plm{g}", name=f"plm{g}")
                for g in range(c.GB)
            ]

            for layer in range(3):
                table = u_full[layer]
                msg_tiles = {}
                for (s, t0, nt) in CALLS:
                    m = msgp.tile([P, c.CALL_TILES, D], f32, tag="msg")
                    nc.gpsimd.dma_gather(
                        m[:, 0:nt, :],
                        table[s][:, :],
                        idx_sb[:, t0 * 8 : (t0 + nt) * 8],
                        nt * P, nt * P, D, elem_step=D,
                        single_packet=False,
                    )
                    for j in range(nt):
                        msg_tiles[t0 + j] = (m, j)
                for (s, d, t0, nt, first) in CELLS:
                    zp = psp.tile([P, D], f32, tag="ps")
                    for j in range(nt):
                        m, col = msg_tiles[t0 + j]
                        S = sp.tile([P, P], f32, tag="S")
                        nc.vector.tensor_scalar(
                            S[:], iota_f_sb[:],
                            ids_sb[:, t0 + j : t0 + j + 1], None, OP.is_equal,
                        )
                        nc.tensor.matmul(
                            zp[:], S[:], m[:, col, 0:D],
                            start=(j == 0), stop=(j == nt - 1),
                        )
                    if first:
                        nc.vector.tensor_copy(z_acc[:, d * D : (d + 1) * D], zp[:])
                    else:
                        nc.vector.tensor_add(
                            z_acc[:, d * D : (d + 1) * D],
                            z_acc[:, d * D : (d + 1) * D], zp[:],
                        )
                for d in range(DBL):
                    zsc = wp.tile([P, D], f32, tag="zsc")
                    nc.vector.tensor_scalar(
                        zsc[:], z_acc[:, d * D : (d + 1) * D],
                        dinv[:, d : d + 1], None, OP.mult,
                    )
                    tp = psp.tile([D, P], f32, tag="ps")
                    nc.tensor.transpose(tp[:], zsc[:], ident[:])
                    wT = wp.tile([D, P], f32, tag="wT")
                    nc.vector.tensor_copy(wT[:], tp[:])
                    op = psp.tile([D, P], f32, tag="ps")
                    nc.tensor.matmul(op[:], W_sb[layer][:], wT[:], start=True, stop=True)
                    oT = wp.tile([D, P], f32, tag="oT")
                    nc.scalar.activation(oT[:], op[:], AF.Relu, bias=b_sb[layer][:])
                    bp = psp.tile([P, D], f32, tag="ps")
                    nc.tensor.transpose(bp[:], oT[:], ident[0:D, 0:D])
                    if layer < 2:
                        ut = wp.tile([P, D], f32, tag="ut")
                        nc.vector.tensor_scalar(
                            ut[:], bp[:], dinv[:, d : d + 1], None, OP.mult
                        )
                        rows = min(c.NLP, (d + 1) * P) - d * P
                        nc.sync.dma_start(
                            u_loc[layer + 1][d * P : d * P + rows, 0:D],
                            ut[0:rows, :],
                        )
                        qend = [(qs + qr) // P for qs, qr in zip(c.qstart, c.qrows)]
                        if d + 1 in qend:
                            q = qend.index(d + 1)
                            nc.gpsimd.collective_compute(
                                "AllGather", OP.bypass, replica_groups=rg,
                                ins=[u_loc[layer + 1][c.qstart[q] : c.qstart[q] + c.qrows[q], :]],
                                outs=[u_full[layer + 1][q][:, :]],
                            )
                    else:
                        # ---- pooling ----
                        xf = wp.tile([P, D + 1], f32, tag="xf")
                        nc.vector.tensor_copy(xf[:, 0:D], bp[:])
                        nc.vector.memset(xf[:, D : D + 1], 1.0)
                        for g in range(c.GB):
                            Sg = sp.tile([P, P], f32, tag="Sg")
                            nc.vector.tensor_scalar(
                                Sg[:], iota_f_sb[:], float(g * P),
                                g_ids_sb[:, d : d + 1], OP.add, OP.is_equal,
                            )
                            nc.tensor.matmul(
                                pool_sum_ps[g][:], Sg[:], xf[:, 0 : D + 1],
                                start=(d == 0), stop=(d == DBL - 1),
                            )
                        # graph-id row broadcast to [D, P] via PE
                        t1 = psp.tile([1, P], f32, tag="ps")
                        nc.tensor.transpose(
                            t1[:], g_ids_sb[:, d : d + 1], ident[:]
                        )
                        t1s = wp.tile([1, P], f32, tag="t1s")
                        nc.vector.tensor_copy(t1s[:], t1[:])
                        gb_ps = psp.tile([D, P], f32, tag="ps")
                        nc.tensor.matmul(
                            gb_ps[:], ones_row[:], t1s[:], start=True, stop=True
                        )
                        gdb = wp.tile([D, P], f32, tag="gdb")
                        nc.vector.tensor_copy(gdb[:], gb_ps[:])
                        mscan = wp.tile([D, P], f32, tag="mscan")
                        nc.vector.tensor_copy(mscan[:], oT[:])
                        sh = 1
                        while sh < P:
                            msk = wp.tile([D, P], f32, tag="msk")
                            nc.vector.tensor_tensor(
                                msk[:, sh:P], gdb[:, sh:P], gdb[:, 0 : P - sh],
                                OP.is_equal,
                            )
                            tmp = wp.tile([D, P], f32, tag="tmpscan")
                            nc.vector.tensor_tensor(
                                tmp[:, sh:P], mscan[:, 0 : P - sh], msk[:, sh:P],
                                OP.mult,
                            )
                            nc.vector.tensor_tensor(
                                mscan[:, sh:P], mscan[:, sh:P], tmp[:, sh:P], OP.max
                            )
                            sh *= 2
                        if pool_carry["g"] is not None:
                            cmask = wp.tile([D, P], f32, tag="cmask")
                            nc.vector.tensor_scalar(
                                cmask[:], gdb[:], pool_carry["g"][:, 0:1], None,
                                OP.is_equal,
                            )
                            nc.vector.tensor_scalar(
                                cmask[:], cmask[:], pool_carry["v"][:, 0:1], None,
                                OP.mult,
                            )
                            nc.vector.tensor_tensor(
                                mscan[:], mscan[:], cmask[:], OP.max
                            )
                        cg = wp.tile([D, 1], f32, tag="cg", bufs=2)
                        cv = wp.tile([D, 1], f32, tag="cv", bufs=2)
                        nc.vector.tensor_copy(cg[:], gdb[:, P - 1 : P])
                        nc.vector.tensor_copy(cv[:], mscan[:, P - 1 : P])
                        pool_carry = {"g": cg, "v": cv}
                        sc_ps = psp.tile([P, D], f32, tag="ps")
                        nc.tensor.transpose(sc_ps[:], mscan[:], ident[0:D, 0:D])
                        scT = wp.tile([P, D], f32, tag="scT")
                        nc.vector.tensor_copy(scT[:], sc_ps[:])
                        for g in range(c.GB):
                            Se = sp.tile([P, P], f32, tag="Sg")
                            nc.vector.tensor_scalar(
                                Se[:], iota_f_sb[:], float(g * P),
                                g_ids_sb[:, d : d + 1], OP.add, OP.is_equal,
                            )
                            nc.vector.tensor_scalar(
                                Se[:], Se[:], is_end_sb[:, d : d + 1], None, OP.mult
                            )
                            nc.tensor.matmul(
                                pool_max_ps[g][:], Se[:], scT[:],
                                start=(d == 0), stop=(d == DBL - 1),
                            )


            # ---------- pool exchange + classifier ---------------------------
            pl = wp.tile([P, c.GB * PCOLS], f32, tag="pl")
            for g in range(c.GB):
                o = g * PCOLS
                nc.vector.tensor_copy(pl[:, o : o + D + 1], pool_sum_ps[g][:])
                nc.vector.tensor_copy(
                    pl[:, o + D + 1 : o + 2 * D + 1], pool_max_ps[g][:]
                )
                nc.vector.memset(pl[:, o + 2 * D + 1 : o + PCOLS], 0.0)
            nc.sync.dma_start(pool_loc_d[:, :], pl[:])
            nc.gpsimd.collective_compute(
                "AllGather", OP.bypass, replica_groups=rg,
                ins=[pool_loc_d[:, :]], outs=[pool_all_d[:, :]],
            )
            comb = wp.tile([P, c.GB * PCOLS], f32, tag="comb")
            nc.vector.memset(comb[:], 0.0)
            for r in range(c.NC):
                pr = wp.tile([P, c.GB * PCOLS], f32, tag="pr")
                nc.sync.dma_start(pr[:], pool_all_d[r * P : (r + 1) * P, :])
                for g in range(c.GB):
                    o = g * PCOLS
                    nc.vector.tensor_add(
                        comb[:, o : o + D + 1], comb[:, o : o + D + 1],
                        pr[:, o : o + D + 1],
                    )
                    nc.vector.tensor_tensor(
                        comb[:, o + D + 1 : o + 2 * D + 1],
                        comb[:, o + D + 1 : o + 2 * D + 1],
                        pr[:, o + D + 1 : o + 2 * D + 1], OP.max,
                    )
            hT = wp.tile([2 * D, c.GB * P], f32, tag="hT")
            for g in range(c.GB):
                o = g * PCOLS
                cnt = wp.tile([P, 1], f32, tag="cnt")
                nc.vector.tensor_scalar(
                    cnt[:], comb[:, o + D : o + D + 1], 1.0, None, OP.max
                )
                rc = wp.tile([P, 1], f32, tag="rc")
                nc.vector.reciprocal(rc[:], cnt[:])
                t2r = wp.tile([P, 1], f32, tag="t2r")
                nc.vector.tensor_tensor(t2r[:], cnt[:], rc[:], OP.mult)
                nc.vector.tensor_scalar(t2r[:], t2r[:], -1.0, 2.0, OP.mult, OP.add)
                nc.vector.tensor_tensor(cnt[:], rc[:], t2r[:], OP.mult)
                mean = wp.tile([P, D], f32, tag="mean")
                nc.vector.tensor_scalar(
                    mean[:], comb[:, o : o + D], cnt[:, 0:1], None, OP.mult
                )
                mps = psp.tile([D, P], f32, tag="ps")
                nc.tensor.transpose(mps[:], mean[:], ident[:])
                nc.vector.tensor_copy(hT[0:D, g * P : (g + 1) * P], mps[:])
                xps = psp.tile([D, P], f32, tag="ps")
                nc.tensor.transpose(
                    xps[:], comb[:, o + D + 1 : o + 2 * D + 1], ident[:]
                )
                nc.vector.tensor_copy(hT[D : 2 * D, g * P : (g + 1) * P], xps[:])
            h1 = psp.tile([D, c.GB * P], f32, tag="ps")
            nc.tensor.matmul(h1[:], Wc1_sb[:], hT[:], start=True, stop=True)
            h1s = wp.tile([D, c.GB * P], f32, tag="h1s")
            nc.scalar.activation(h1s[:], h1[:], AF.Relu, bias=bc1_sb[:])
            lg = psp.tile([2, c.GB * P], f32, tag="ps")
            nc.tensor.matmul(lg[:], Wc2_sb[:], h1s[:], start=True, stop=True)
            lgs = wp.tile([2, c.GB * P], f32, tag="lgs")
            nc.scalar.activation(lgs[:], lg[:], AF.Identity, bias=bc2_sb[:])
            for g in range(c.GB):
                lt = psp.tile([P, 2], f32, tag="ps")
                nc.tensor.transpose(
                    lt[:], lgs[:, g * P : (g + 1) * P], ident[0:2, 0:2]
                )
                lts = wp.tile([P, 2], f32, tag="lts")
                nc.vector.tensor_copy(lts[:], lt[:])
                rows = min(c.G, (g + 1) * P) - g * P
                nc.sync.dma_start(logits_d[g * P : g * P + rows, :], lts[0:rows, :])

    nc.compile()
    return nc


def make_in_maps(cfg, data, inputs):
    shared = {
        "emb_table": np.asarray(inputs["emb_table"], dtype=np.float32),
        "Wc1": np.asarray(inputs["Wc1"], dtype=np.float32),
        "bc1": np.asarray(inputs["bc1"], dtype=np.float32),
        "Wc2": np.asarray(inputs["Wc2"], dtype=np.float32),
        "bc2": np.asarray(inputs["bc2"], dtype=np.float32),
    }
    for i in range(3):
        shared[f"W{i}"] = np.asarray(inputs[f"W{i}"], dtype=np.float32)
        shared[f"b{i}"] = np.asarray(inputs[f"b{i}"], dtype=np.float32)
    in_maps = []
    for ci in range(cfg.NC):
        m = dict(shared)
        for k, v in data.items():
            m[k] = v[ci]
        in_maps.append(m)
    return in_maps


def kernel(**inputs):
    from concourse.bass_utils import run_bass_kernel_spmd

    x_tokens = np.asarray(inputs["x_tokens"])
    edge_index = np.asarray(inputs["edge_index"])
    batch = np.asarray(inputs["batch"])
    N = x_tokens.shape[0]
    E = edge_index.shape[1]
    V = np.asarray(inputs["emb_table"]).shape[0]
    G = 256
    cfg = Cfg(N, E, V, G)

    meta, data = _preprocess(cfg, x_tokens, edge_index, batch)
    nc = build_program(cfg, meta)
    in_maps = make_in_maps(cfg, data, inputs)
    res = run_bass_kernel_spmd(nc, in_maps, core_ids=list(range(cfg.NC)))
    return np.asarray(res.results[0]["logits"])

